# revision 3
# baseline (speedup 1.0000x reference)
"""Trainium2 Bass kernel for nn_DigitConvolutionalModel.

Model: x(B,784) -> reshape 28x28 -> 3x3 valid cross-correlation (kernel is an
input) -> flatten 676 -> Linear(676,128)+ReLU -> Linear(128,10).

Strategy:
  * Fold the 3x3 conv into the first linear layer on the host: the conv is a
    linear map, so h = relu(x @ W1eff.T + b1) with W1eff (128, 784) built by
    scattering conv_w-weighted copies of w1 onto the 28x28 grid. The device
    kernel is then a plain 2-layer MLP over 784 features.
  * Pure data parallelism: batch 65536 split as 8192 rows per NeuronCore,
    weights replicated.
  * Activations are shipped feature-major and fp16 (PE runs fp16 at full
    rate; the per-core HBM ceiling ~358 GB/s is the roofline, so halving
    bytes halves the kernel time; measured error ~5e-4 of scale).
    The kernel computes logits^T = w2 @ relu(W1eff @ x^T + b1) + b2 and the
    host transposes the gathered (10, B) result back.
  * x is shipped packed per DMA block with each partition's data fully
    contiguous in HBM, so a block load is 112 descriptors of 14 KB each
    (4 KB descriptors leave the 16 SDMA engines descriptor-bound at
    ~240 GB/s; large descriptors reach the HBM limit).
  * Blocks of 1024 batch rows: block compute (~3 us) tracks block DMA
    (~4.6 us) closely enough that the PE never idles past the ~3.4 us HAM
    window, keeping the clock at 2.4 GHz.
"""

from contextlib import ExitStack

import numpy as np

B = 65536
H = W = 28
K = 3
CH = CW = 26
FEAT = H * W          # 784
HID = 128
OUT = 10
NCORES = 8
BC = B // NCORES      # 8192 rows per core

KC = 112              # contraction-chunk partition size
KCH = 7               # chunks: 7 * 112 = 784
NT = 512              # max batch rows per compute tile (one PSUM bank fp32)
XB = 1024             # batch rows per DMA block

VARIANT = "f16"

_NC_CACHE = {}


def _blocks(bc):
    return [min(XB, bc - o) for o in range(0, bc, XB)]


def _tiles(xb):
    out, t0 = [], 0
    while t0 < xb:
        nt = min(NT, xb - t0)
        out.append((t0, nt))
        t0 += nt
    return out


def _dtypes(variant):
    import concourse.mybir as mybir

    f32 = mybir.dt.float32
    if variant == "f32":
        return f32, f32
    if variant == "bf16":
        return mybir.dt.bfloat16, mybir.dt.bfloat16
    if variant == "f16":
        return mybir.dt.float16, mybir.dt.float16
    raise ValueError(variant)


def _build_nc(bc, variant):
    from concourse import bacc
    import concourse.mybir as mybir
    import concourse.tile as tile

    f32 = mybir.dt.float32
    wdt, xdt = _dtypes(variant)
    blocks = _blocks(bc)

    nc = bacc.Bacc(
        "TRN2",
        target_bir_lowering=False,
        debug=False,
        enable_asserts=False,
        num_devices=NCORES,
    )
    # [112, 7*bc] with per-block column groups: block b at columns
    # [7*off_b, 7*(off_b+xb)), laid out [chunk, row] so each partition's
    # block data is one contiguous run -> one big DMA descriptor per
    # partition per block
    xT = nc.dram_tensor("xT", [KC, KCH * bc], xdt, kind="ExternalInput").ap()
    w1t = nc.dram_tensor("w1t", [KC, KCH, HID], wdt, kind="ExternalInput").ap()
    b1 = nc.dram_tensor("b1", [HID, 1], f32, kind="ExternalInput").ap()
    w2t = nc.dram_tensor("w2t", [HID, OUT], wdt, kind="ExternalInput").ap()
    b2 = nc.dram_tensor("b2", [OUT, 1], f32, kind="ExternalInput").ap()
    outT = nc.dram_tensor("outT", [OUT, bc], f32, kind="ExternalOutput").ap()

    with ExitStack() as ctx:
        tc = ctx.enter_context(tile.TileContext(nc))
        wpool = ctx.enter_context(tc.tile_pool(name="w", bufs=1))
        xpool = ctx.enter_context(tc.tile_pool(name="x", bufs=4))
        hpool = ctx.enter_context(tc.tile_pool(name="h", bufs=3))
        opool = ctx.enter_context(tc.tile_pool(name="o", bufs=2))
        p1pool = ctx.enter_context(tc.tile_pool(name="p1", bufs=4, space="PSUM"))
        p2pool = ctx.enter_context(tc.tile_pool(name="p2", bufs=2, space="PSUM"))

        # x-block loads own the sync HWDGE ring from instruction zero;
        # weights + outputs ride the scalar ring so a waiting output store
        # never head-of-line-blocks an x load
        w1s = wpool.tile([KC, KCH, HID], wdt)
        b1s = wpool.tile([HID, 1], f32)
        w2s = wpool.tile([HID, OUT], wdt)
        b2s = wpool.tile([OUT, 1], f32)

        xs_list = []
        off = 0
        for blk, xb in enumerate(blocks):
            xs = xpool.tile([KC, KCH * xb], xdt, tag="xs", name=f"xs_{blk}")
            nc.sync.dma_start(xs[:], xT[:, KCH * off : KCH * (off + xb)])
            xs_list.append(xs)
            off += xb

        nc.scalar.dma_start(w1s[:], w1t[:])
        nc.scalar.dma_start(b1s[:], b1[:])
        nc.scalar.dma_start(w2s[:], w2t[:])
        nc.scalar.dma_start(b2s[:], b2[:])

        add = mybir.AluOpType.add
        mx = mybir.AluOpType.max

        off = 0
        for blk, xb in enumerate(blocks):
            xs = xs_list[blk]
            os_ = opool.tile([OUT, xb], f32, tag="os", name=f"os_{blk}")
            # tile-major: each 512-tile's 7 chunk-matmuls run back-to-back
            # (K-contiguous), its epilogue overlaps the next tile's matmuls
            for i, (t0, nt) in enumerate(_tiles(xb)):
                p1 = p1pool.tile([HID, nt], f32, tag="p1", name=f"p1_{blk}_{i}")
                for c in range(KCH):
                    nc.tensor.matmul(
                        p1[:],
                        w1s[:, c, :],
                        xs[:, c * xb + t0 : c * xb + t0 + nt],
                        start=(c == 0),
                        stop=(c == KCH - 1),
                    )
                hs = hpool.tile([HID, nt], xdt, tag="hs", name=f"hs_{blk}_{i}")
                nc.vector.tensor_scalar(hs[:], p1[:], b1s[:], 0.0, add, mx)
                p2 = p2pool.tile([OUT, nt], f32, tag="p2", name=f"p2_{blk}_{i}")
                nc.tensor.matmul(p2[:], w2s[:], hs[:], start=True, stop=True)
                nc.vector.tensor_scalar_add(os_[:, t0 : t0 + nt], p2[:], b2s[:])
            nc.scalar.dma_start(outT[:, off : off + xb], os_[:])
            off += xb

    nc.compile()
    return nc


def get_nc(bc=BC, variant=VARIANT):
    key = (bc, variant)
    if key not in _NC_CACHE:
        _NC_CACHE[key] = _build_nc(bc, variant)
    return _NC_CACHE[key]


def _np_wdt(variant):
    if variant == "bf16":
        import ml_dtypes

        return ml_dtypes.bfloat16
    if variant == "f16":
        return np.float16
    return np.float32


def _pack_xT(shard, blocks, wnp):
    """[bc, 784] row-major shard -> [112, 7*bc] per-block-contiguous."""
    bc = shard.shape[0]
    parts = []
    off = 0
    for xb in blocks:
        sub = shard[off : off + xb]  # [xb, 784]
        # [xb, 7, 112] -> [112, 7, xb] -> [112, 7*xb]
        parts.append(sub.reshape(xb, KCH, KC).transpose(2, 1, 0).reshape(KC, KCH * xb))
        off += xb
    return np.ascontiguousarray(np.concatenate(parts, axis=1)).astype(wnp, copy=False)


def _host_prep(x, conv_w, w1, b1, w2, b2, variant):
    """Fold conv into layer-1 weights and lay out per-core device inputs."""
    x = np.asarray(x, dtype=np.float32)
    conv_w = np.asarray(conv_w, dtype=np.float32)
    w1 = np.asarray(w1, dtype=np.float32)
    b1 = np.asarray(b1, dtype=np.float32)
    w2 = np.asarray(w2, dtype=np.float32)
    b2 = np.asarray(b2, dtype=np.float32)

    w1_img = w1.reshape(HID, CH, CW)
    w1eff = np.zeros((HID, H, W), dtype=np.float32)
    for di in range(K):
        for dj in range(K):
            w1eff[:, di : di + CH, dj : dj + CW] += conv_w[di, dj] * w1_img
    w1eff = w1eff.reshape(HID, FEAT)

    wnp = _np_wdt(variant)
    # [784,128] -> [7,112,128] -> [112,7,128] so chunk c partition p holds
    # feature c*112+p
    w1t_host = np.ascontiguousarray(
        w1eff.T.reshape(KCH, KC, HID).transpose(1, 0, 2)
    ).astype(wnp)
    b1_host = np.ascontiguousarray(b1.reshape(HID, 1))
    w2t_host = np.ascontiguousarray(w2.T).astype(wnp)
    b2_host = np.ascontiguousarray(b2.reshape(OUT, 1))

    blocks = _blocks(BC)
    xq = x.astype(wnp)
    in_maps = []
    for c in range(NCORES):
        in_maps.append(
            {
                "xT": _pack_xT(xq[c * BC : (c + 1) * BC], blocks, wnp),
                "w1t": w1t_host,
                "b1": b1_host,
                "w2t": w2t_host,
                "b2": b2_host,
            }
        )
    return in_maps


def run(x, conv_w, w1, b1, w2, b2, trace=False, variant=VARIANT):
    from concourse.bass_utils import run_bass_kernel_spmd

    in_maps = _host_prep(x, conv_w, w1, b1, w2, b2, variant)
    nc = get_nc(BC, variant)
    res = run_bass_kernel_spmd(nc, in_maps, list(range(NCORES)), trace=trace)
    outT = np.concatenate([r["outT"] for r in res.results], axis=1)  # [10, B]
    return np.ascontiguousarray(outT.T), res


def kernel(x, conv_w, w1, b1, w2, b2):
    out, _ = run(x, conv_w, w1, b1, w2, b2)
    return out
